# revision 1
# baseline (speedup 1.0000x reference)
"""TransformerXL relative attention on 8 TRN2 NeuronCores (batch-parallel).

v2: all-bf16 matmul pipeline (f32 PSUM accumulation), DMA-xbar attn
transposes, ScalarE reserved for Exp, head-pair row/col packed matmuls.

Per-core (one batch element):
  - load query^T / [mem|query]^T / pos_enc^T (bf16), stream weights (bf16)
  - projections: qcbT/qpbT [hs,q], kT [hs,r], rT [hs,rel], v [r,hs]
  - position logits P[q,rel] per head -> bf16 -> DRAM scratch with padded
    rows (pad = -30000); read back with a skewed affine AP implementing
    TransformerXL rel_shift exactly (masked region reads pad -> exp -> 0)
  - scores psum = content matmul + identity-matmul(P_shifted);
    exp via ScalarE (scale=1/8) with accum_out giving softmax denominators
  - normalize attn (bf16), transpose via DMA-xbar to attnT [r,q],
    PV matmuls (col-packed head pairs), output projection, DMA out.
"""

import sys

if "/opt/trn_rl_repo" not in sys.path:
    sys.path.insert(0, "/opt/trn_rl_repo")

import numpy as np

B, Q, MEM, D, H, S = 8, 512, 512, 1024, 16, 64
R = Q + MEM  # 1024
L = 1536  # padded row pitch of the P scratch buffer (1024 data + 512 pad)
PAD_VAL = -30000.0
NKD = D // 128  # 8 contraction tiles
NI = D // 128  # 8 hs-tiles
NQT = Q // 128  # 4 q-tiles
NRT = R // 128  # 8 r-tiles

_CACHE = {}


def _build_nc():
    import concourse.bass as bass
    import concourse.mybir as mybir
    import concourse.tile as tile
    from concourse import bacc
    from concourse.bass import ds
    from concourse.masks import make_identity

    f32 = mybir.dt.float32
    bf16 = mybir.dt.bfloat16
    AF = mybir.ActivationFunctionType

    nc = bacc.Bacc("TRN2", target_bir_lowering=False)

    qTin = nc.dram_tensor("qT", [D, Q], bf16, kind="ExternalInput")
    refTin = nc.dram_tensor("refT", [D, R], bf16, kind="ExternalInput")
    posTin = nc.dram_tensor("posT", [D, R], bf16, kind="ExternalInput")
    Wq_d = nc.dram_tensor("Wq", [D, D], bf16, kind="ExternalInput")
    Wk_d = nc.dram_tensor("Wk", [D, D], bf16, kind="ExternalInput")
    Wv_d = nc.dram_tensor("Wv", [D, D], bf16, kind="ExternalInput")
    Wr_d = nc.dram_tensor("Wr", [D, D], bf16, kind="ExternalInput")
    Wo_d = nc.dram_tensor("Wo", [D, D], bf16, kind="ExternalInput")
    cb_d = nc.dram_tensor("cb", [128, NI], f32, kind="ExternalInput")
    pb_d = nc.dram_tensor("pb", [128, NI], f32, kind="ExternalInput")
    out_d = nc.dram_tensor("out", [Q, D], f32, kind="ExternalOutput")

    with tile.TileContext(nc) as tc:
        with (
            tc.tile_pool(name="persist", bufs=1) as persist,
            tc.tile_pool(name="dram", bufs=1, space="DRAM") as dram,
        ):
            ident = persist.tile([128, 128], bf16, tag="ident")
            make_identity(nc, ident)
            cb_sb = persist.tile([128, NI], f32, tag="cb")
            pb_sb = persist.tile([128, NI], f32, tag="pb")
            nc.sync.dma_start(out=cb_sb, in_=cb_d[:, :])
            nc.sync.dma_start(out=pb_sb, in_=pb_d[:, :])

            kT = persist.tile([128, NI, R], bf16, tag="kT")
            v_sb = persist.tile([128, NRT, D], bf16, tag="v")
            qcb = persist.tile([128, NI, Q], bf16, tag="qcb")
            qpb = persist.tile([128, NI, Q], bf16, tag="qpb")
            outT = persist.tile([128, NI, Q], bf16, tag="outT")

            Pdram = [
                dram.tile([Q * L], bf16, tag=f"pbuf{h}", name=f"pbuf{h}")
                for h in range(H)
            ]

            with tc.tile_pool(name="rtp", bufs=1) as rtp:
                rT = rtp.tile([128, NI, R], bf16, tag="rT")

                # ============ phase B: projections ============
                with (
                    tc.tile_pool(name="inp", bufs=1) as inp,
                    tc.tile_pool(name="wst", bufs=6) as wst,
                    tc.tile_pool(name="pjB", bufs=8, space="PSUM") as pjB,
                ):
                    # prefetch all activations up front (bf16: 5 MB total)
                    posT = inp.tile([128, NKD, R], bf16, tag="posT")
                    qT_sb = inp.tile([128, NKD, Q], bf16, tag="qTin")
                    refT = inp.tile([128, NKD, R], bf16, tag="refT")
                    for kd in range(NKD):
                        nc.sync.dma_start(
                            out=posT[:, kd, :],
                            in_=posTin[kd * 128 : (kd + 1) * 128, :],
                        )
                    for kd in range(NKD):
                        nc.sync.dma_start(
                            out=qT_sb[:, kd, :],
                            in_=qTin[kd * 128 : (kd + 1) * 128, :],
                        )
                    for kd in range(NKD):
                        nc.sync.dma_start(
                            out=refT[:, kd, :],
                            in_=refTin[kd * 128 : (kd + 1) * 128, :],
                        )

                    def proj(w_dram, rhs_sb_of, n_blocks, emit_out):
                        """for each 512-col block nb, accumulate over kd:
                        psum[i] += w[kd][:, i*128:+128].T @ rhs(kd, nb)."""
                        for nb in range(n_blocks):
                            psums = [
                                pjB.tile([128, 512], f32, tag="pj", name="pj")
                                for _ in range(NI)
                            ]
                            for kd in range(NKD):
                                wt = wst.tile([128, D], bf16, tag="w")
                                nc.gpsimd.dma_start(
                                    out=wt, in_=w_dram[kd * 128 : (kd + 1) * 128, :]
                                )
                                for i in range(NI):
                                    nc.tensor.matmul(
                                        psums[i],
                                        lhsT=wt[:, i * 128 : (i + 1) * 128],
                                        rhs=rhs_sb_of(kd, nb),
                                        start=(kd == 0),
                                        stop=(kd == NKD - 1),
                                    )
                            for i in range(NI):
                                emit_out(i, nb, psums[i])

                    def emit_rT(i, nb, ps):
                        eng = nc.vector if i % 2 else nc.scalar
                        if i % 2:
                            nc.vector.tensor_copy(
                                rT[:, i, ds(nb * 512, 512)], ps)
                        else:
                            nc.scalar.copy(rT[:, i, ds(nb * 512, 512)], ps)

                    proj(Wr_d, lambda kd, nb: posT[:, kd, ds(nb * 512, 512)],
                         2, emit_rT)

                    def emit_q(i, nb, ps):
                        nc.vector.tensor_scalar_add(
                            qcb[:, i, :], ps, cb_sb[:, i : i + 1]
                        )
                        nc.vector.tensor_scalar_add(
                            qpb[:, i, :], ps, pb_sb[:, i : i + 1]
                        )

                    proj(Wq_d, lambda kd, nb: qT_sb[:, kd, :], 1, emit_q)

                    def emit_kT(i, nb, ps):
                        if i % 2:
                            nc.vector.tensor_copy(
                                kT[:, i, ds(nb * 512, 512)], ps)
                        else:
                            nc.scalar.copy(kT[:, i, ds(nb * 512, 512)], ps)

                    proj(Wk_d, lambda kd, nb: refT[:, kd, ds(nb * 512, 512)],
                         2, emit_kT)

                    # v natural [r, hs]
                    for nb in range(2):
                        psums = [
                            pjB.tile([128, 512], f32, tag="pj", name="pj")
                            for _ in range(NRT)
                        ]
                        for kd in range(NKD):
                            wt = wst.tile([128, D], bf16, tag="w")
                            nc.gpsimd.dma_start(
                                out=wt, in_=Wv_d[kd * 128 : (kd + 1) * 128, :]
                            )
                            for rt in range(NRT):
                                nc.tensor.matmul(
                                    psums[rt],
                                    lhsT=refT[:, kd, ds(rt * 128, 128)],
                                    rhs=wt[:, ds(nb * 512, 512)],
                                    start=(kd == 0),
                                    stop=(kd == NKD - 1),
                                )
                        for rt in range(NRT):
                            if rt % 2:
                                nc.vector.tensor_copy(
                                    v_sb[:, rt, ds(nb * 512, 512)], psums[rt]
                                )
                            else:
                                nc.scalar.copy(
                                    v_sb[:, rt, ds(nb * 512, 512)], psums[rt]
                                )

                # ====== phases C/D: software-pipelined over head pairs ======
                import concourse.bass as bass_mod

                with (
                    tc.tile_pool(name="pst", bufs=8) as pstp,
                    tc.tile_pool(name="psh", bufs=14) as pshp,
                    tc.tile_pool(name="attn", bufs=4) as attnp,
                    tc.tile_pool(name="attnT", bufs=4) as attnTp,
                    tc.tile_pool(name="den", bufs=8) as denp,
                    tc.tile_pool(name="ppsum", bufs=2, space="PSUM") as ppsum,
                    tc.tile_pool(name="scp", bufs=2, space="PSUM") as scp,
                    tc.tile_pool(name="pvp", bufs=2, space="PSUM") as pvp,
                ):
                    NP = H // 2  # 8 head pairs

                    # one-time pad fill of every Pdram row tail: rows qi of
                    # head h, cols [R, L) = PAD_VAL so the skewed read pulls
                    # pad exactly on masked (r > q + MEM) entries.
                    padsrc = pstp.tile([128, L - R], bf16, tag="padsrc",
                                       name="padsrc")
                    nc.vector.memset(padsrc, PAD_VAL)
                    for h in range(H):
                        for qt in range(NQT):
                            pad_ap = bass_mod.AP(
                                tensor=Pdram[h].tensor,
                                offset=Pdram[h].offset + qt * 128 * L + R,
                                ap=[[L, 128], [1, L - R]],
                            )
                            nc.gpsimd.dma_start(out=pad_ap, in_=padsrc)

                    def emit_C(j):
                        """position logits P for heads 2j,2j+1 -> Pdram."""
                        i_h = j
                        for qt in range(NQT):
                            psts = []
                            for hh in range(2):
                                pst = pstp.tile([128, R], bf16, tag="pst",
                                                name="pst")
                                psts.append(pst)
                            for rb in range(2):
                                pps = []
                                for hh in range(2):
                                    off = hh * 64
                                    pp = ppsum.tile([128, 512], f32,
                                                    tag="pp", name="pp")
                                    nc.tensor.matmul(
                                        pp,
                                        lhsT=qpb[off : off + 64, i_h,
                                                 ds(qt * 128, 128)],
                                        rhs=rT[off : off + 64, i_h,
                                               ds(rb * 512, 512)],
                                        start=True,
                                        stop=True,
                                        tile_position=(off, 0),
                                    )
                                    pps.append(pp)
                                for hh in range(2):
                                    nc.vector.tensor_copy(
                                        psts[hh][:, ds(rb * 512, 512)],
                                        pps[hh],
                                    )
                            for hh in range(2):
                                h = 2 * j + hh
                                wr_ap = bass_mod.AP(
                                    tensor=Pdram[h].tensor,
                                    offset=Pdram[h].offset + qt * 128 * L,
                                    ap=[[L, 128], [1, R]],
                                )
                                nc.sync.dma_start(out=wr_ap, in_=psts[hh])

                    def emit_PSH(j):
                        """prefetch skewed P reads for pair j."""
                        pshs = {}
                        for qt in range(NQT):
                            for hh in range(2):
                                h = 2 * j + hh
                                psh = pshp.tile([128, R], bf16, tag="psh",
                                                name="psh")
                                rd_ap = bass_mod.AP(
                                    tensor=Pdram[h].tensor,
                                    offset=Pdram[h].offset
                                    + qt * 128 * (L - 1) + 511,
                                    ap=[[L - 1, 128], [1, R]],
                                )
                                nc.sync.dma_start(out=psh, in_=rd_ap)
                                pshs[(hh, qt)] = psh
                        return pshs

                    def emit_D(j, pshs):
                        """scores+exp+normalize+xbar for pair j; returns aT."""
                        i_h = j
                        apair = {}
                        a32 = {}
                        for hh in range(2):
                            apair[hh] = attnp.tile([128, NQT, R], bf16,
                                                   tag="attn", name="attn")
                            a32[hh] = attnTp.tile([128, NQT * NRT, 128], bf16,
                                                  tag="attnT", name="attnT")
                        for qt in range(NQT):
                            scs = []
                            for hh in range(2):
                                off = hh * 64
                                sc = scp.tile([128, 1024], f32, tag="sc",
                                              name="sc")
                                for rb in range(2):
                                    nc.tensor.matmul(
                                        sc[:, ds(rb * 512, 512)],
                                        lhsT=qcb[off : off + 64, i_h,
                                                 ds(qt * 128, 128)],
                                        rhs=kT[off : off + 64, i_h,
                                               ds(rb * 512, 512)],
                                        start=True,
                                        stop=False,
                                        tile_position=(off, 0),
                                    )
                                scs.append((hh, sc))
                            for hh, sc in scs:
                                for rb in range(2):
                                    nc.tensor.matmul(
                                        sc[:, ds(rb * 512, 512)],
                                        lhsT=ident,
                                        rhs=pshs[(hh, qt)][:, ds(rb * 512, 512)],
                                        start=False,
                                        stop=True,
                                        skip_group_check=True,
                                    )
                            for hh, sc in scs:
                                attn = apair[hh][:, qt, :]
                                den = denp.tile([128, 4], f32, tag="den",
                                                name="den")
                                nc.scalar.activation(
                                    attn,
                                    sc,
                                    AF.Exp,
                                    scale=0.125,
                                    accum_out=den[:, 0:1],
                                )
                                nc.vector.reciprocal(den[:, 1:2], den[:, 0:1])
                                nc.vector.tensor_scalar_mul(
                                    attn, attn, den[:, 1:2]
                                )
                                if qt == 1:
                                    nc.scalar.dma_start_transpose(
                                        a32[hh][:, 0 : 2 * NRT, :],
                                        apair[hh][:, 0:2, :],
                                    )
                                elif qt == NQT - 1:
                                    nc.scalar.dma_start_transpose(
                                        a32[hh][:, 2 * NRT : 4 * NRT, :],
                                        apair[hh][:, 2:4, :],
                                    )
                        aT = {}
                        for hh in range(2):
                            aT[hh] = a32[hh].rearrange(
                                "p (qt rt) q -> p qt rt q", rt=NRT
                            )
                        return aT

                    def emit_PV(j, aT):
                        pv = pvp.tile([128, 512], f32, tag="pv", name="pv")
                        for rt in range(NRT):
                            for hh in range(2):
                                h = 2 * j + hh
                                off = hh * 64
                                nc.tensor.matmul(
                                    pv[off : off + 64, :],
                                    lhsT=v_sb[:, rt, ds(h * 64, 64)],
                                    rhs=aT[hh][:, :, rt, :],
                                    start=(rt == 0),
                                    stop=(rt == NRT - 1),
                                    tile_position=(0, off),
                                )
                        nc.vector.tensor_copy(outT[:, j, :], pv)

                    # pipeline: C two pairs ahead, PV one pair behind
                    emit_C(0)
                    emit_C(1)
                    pend = {}
                    pshq = {0: emit_PSH(0)}
                    for j in range(NP):
                        if j + 1 < NP:
                            pshq[j + 1] = emit_PSH(j + 1)
                        pend[j] = emit_D(j, pshq.pop(j))
                        if j + 2 < NP:
                            emit_C(j + 2)
                        if j - 1 in pend:
                            emit_PV(j - 1, pend.pop(j - 1))
                    emit_PV(NP - 1, pend.pop(NP - 1))

            # ============ output projection ============
            with (
                tc.tile_pool(name="wo", bufs=1) as wop,
                tc.tile_pool(name="ost", bufs=3) as ostp,
                tc.tile_pool(name="opj", bufs=2, space="PSUM") as opj,
            ):
                Wo_sb = wop.tile([128, NI, D], bf16, tag="Wo")
                nc.sync.dma_start(
                    out=Wo_sb, in_=Wo_d.rearrange("(i p) d -> p i d", p=128)
                )
                for qt in range(NQT):
                    for db in range(2):
                        op = opj.tile([128, 512], f32, tag="op", name="op")
                        for i in range(NI):
                            nc.tensor.matmul(
                                op,
                                lhsT=outT[:, i, ds(qt * 128, 128)],
                                rhs=Wo_sb[:, i, ds(db * 512, 512)],
                                start=(i == 0),
                                stop=(i == NI - 1),
                            )
                        ot = ostp.tile([128, 512], f32, tag="ot", name="ot")
                        nc.vector.tensor_copy(ot, op)
                        nc.sync.dma_start(
                            out=out_d[
                                qt * 128 : (qt + 1) * 128,
                                db * 512 : (db + 1) * 512,
                            ],
                            in_=ot,
                        )

    return nc


def _get_nc():
    if "nc" not in _CACHE:
        nc = _build_nc()
        if not nc.is_finalized():
            nc.finalize()
        _CACHE["nc"] = nc
    return _CACHE["nc"]


def _prep_in_maps(inputs):
    import ml_dtypes

    bf = ml_dtypes.bfloat16
    q = np.asarray(inputs["query_seqs"], dtype=np.float32)
    mem = np.asarray(inputs["memory_seqs"], dtype=np.float32)
    pos = np.asarray(inputs["positional_encoding"], dtype=np.float32)
    Wq = np.asarray(inputs["Wq"], dtype=np.float32).reshape(D, D).astype(bf)
    Wk = np.asarray(inputs["Wk"], dtype=np.float32).reshape(D, D).astype(bf)
    Wv = np.asarray(inputs["Wv"], dtype=np.float32).reshape(D, D).astype(bf)
    Wr = np.asarray(inputs["Wr"], dtype=np.float32).reshape(D, D).astype(bf)
    Wo = np.asarray(inputs["Wo"], dtype=np.float32).reshape(D, D).astype(bf)
    cb = np.ascontiguousarray(
        np.asarray(inputs["content_bias"], dtype=np.float32)
        .reshape(D).reshape(NI, 128).T
    )
    pb = np.ascontiguousarray(
        np.asarray(inputs["position_bias"], dtype=np.float32)
        .reshape(D).reshape(NI, 128).T
    )
    posT = np.ascontiguousarray(pos.T).astype(bf)

    in_maps = []
    for b in range(B):
        refT = np.ascontiguousarray(
            np.concatenate([mem[b], q[b]], axis=0).T
        ).astype(bf)
        qT = np.ascontiguousarray(q[b].T).astype(bf)
        in_maps.append(
            dict(
                qT=qT, refT=refT, posT=posT,
                Wq=Wq, Wk=Wk, Wv=Wv, Wr=Wr, Wo=Wo, cb=cb, pb=pb,
            )
        )
    return in_maps


def run_spmd(inputs, **kwargs):
    """Run on 8 cores; returns (output [B,Q,D], BassKernelResults)."""
    from concourse.bass_utils import run_bass_kernel_spmd

    nc = _get_nc()
    in_maps = _prep_in_maps(inputs)
    res = run_bass_kernel_spmd(nc, in_maps, core_ids=list(range(B)), **kwargs)
    out = np.stack([r["out"] for r in res.results], axis=0).astype(np.float32)
    return out, res


def kernel(**inputs) -> np.ndarray:
    out, _ = run_spmd(inputs)
    return out



# revision 9
# speedup vs baseline: 1.1367x; 1.1367x over previous
"""TransformerXL relative attention on 8 TRN2 NeuronCores (batch-parallel).

v3: dense-pipeline rewrite of v2.
  - position-logit phase (C) interleaved with kT/v projections so TensorE
    stays warm (no HAM re-throttle) and P-scratch DMA overlaps matmuls
  - P scratch pitch L=1152: 128-col pad tail lives inside each pst tile
    (memset once per pool buffer) and is written by the same DMA as the
    data -> no separate pad-fill DMAs
  - masked-width trimming: per q-tile qt only cols [0, 640+128*qt) of the
    shifted scores are real; P writes [c0,1152), skewed reads [0,W),
    exp/normalize/transpose trimmed to W; fully-masked attnT blocks are
    memset to zero instead of transposed
  - engine split: Scalar=exp(+accum), Vector=normalize/memsets/copies,
    GpSimd=psh reads+copies, Sync=P writes+attn transposes, weights on
    GpSimd; deep psh prefetch (2 head-pairs ahead)
"""

import sys

if "/opt/trn_rl_repo" not in sys.path:
    sys.path.insert(0, "/opt/trn_rl_repo")

import numpy as np

B, Q, MEM, D, H, S = 8, 512, 512, 1024, 16, 64
R = Q + MEM  # 1024
L = R + 128  # 1152: padded row pitch of P scratch (1024 data + 128 pad)
PAD_VAL = -30000.0
NKD = D // 128  # 8 contraction tiles
NI = D // 128  # 8 hs-tiles
NQT = Q // 128  # 4 q-tiles
NRT = R // 128  # 8 r-tiles
NP = H // 2  # 8 head pairs
# per q-tile qt: shifted cols [0, W) are live; P data cols [c0, 1024) used
W_QT = [640, 768, 896, 1024]
C0_QT = [384, 256, 128, 0]

_CACHE = {}


def _build_nc():
    import concourse.bass as bass_mod
    import concourse.mybir as mybir
    import concourse.tile as tile
    from concourse import bacc
    from concourse.bass import ds
    from concourse.masks import make_identity

    f32 = mybir.dt.float32
    bf16 = mybir.dt.bfloat16
    AF = mybir.ActivationFunctionType

    nc = bacc.Bacc("TRN2", target_bir_lowering=False)

    qTin = nc.dram_tensor("qT", [D, Q], bf16, kind="ExternalInput")
    refTin = nc.dram_tensor("refT", [D, R], bf16, kind="ExternalInput")
    posTin = nc.dram_tensor("posT", [D, R], bf16, kind="ExternalInput")
    Wq_d = nc.dram_tensor("Wq", [D, D], bf16, kind="ExternalInput")
    Wk_d = nc.dram_tensor("Wk", [D, D], bf16, kind="ExternalInput")
    Wv_d = nc.dram_tensor("Wv", [D, D], bf16, kind="ExternalInput")
    Wr_d = nc.dram_tensor("Wr", [D, D], bf16, kind="ExternalInput")
    Wo_d = nc.dram_tensor("Wo", [D, D], bf16, kind="ExternalInput")
    cb_d = nc.dram_tensor("cb", [128, NI], f32, kind="ExternalInput")
    pb_d = nc.dram_tensor("pb", [128, NI], f32, kind="ExternalInput")
    out_d = nc.dram_tensor("out", [Q, D], f32, kind="ExternalOutput")


    def ecopy(eng, out, in_):
        if eng is nc.scalar:
            eng.copy(out, in_)
        else:
            eng.tensor_copy(out, in_)

    with tile.TileContext(nc) as tc:
        with (
            tc.tile_pool(name="persist", bufs=1) as persist,
            tc.tile_pool(name="dram", bufs=1, space="DRAM") as dram,
        ):
            ident = persist.tile([128, 128], bf16, tag="ident")
            make_identity(nc, ident)
            cb_sb = persist.tile([128, NI], f32, tag="cb")
            pb_sb = persist.tile([128, NI], f32, tag="pb")
            nc.sync.dma_start(out=cb_sb, in_=cb_d[:, :])
            nc.sync.dma_start(out=pb_sb, in_=pb_d[:, :])

            kT = persist.tile([128, NI, R], bf16, tag="kT")
            v_sb = persist.tile([128, NRT, D], bf16, tag="v")
            qcb = persist.tile([128, NI, Q], bf16, tag="qcb")
            qpb = persist.tile([128, NI, Q], bf16, tag="qpb")
            outT = persist.tile([128, NI, Q], bf16, tag="outT")

            Pdram = [
                dram.tile([Q * L], bf16, tag=f"pbuf{h}", name=f"pbuf{h}")
                for h in range(H)
            ]

            with tc.tile_pool(name="rtp", bufs=1) as rtp:
                rT = rtp.tile([128, NI, R], bf16, tag="rT")

                # ============ inputs + pre-phase: rT, qcb/qpb ============
                with (
                    tc.tile_pool(name="inp", bufs=1) as inp,
                    tc.tile_pool(name="pj", bufs=4, space="PSUM") as pj,
                    tc.tile_pool(name="pstp", bufs=8) as pstp,
                    tc.tile_pool(name="ppsum", bufs=4, space="PSUM") as ppsum,
                ):
                    posT = inp.tile([128, NKD, R], bf16, tag="posT")
                    qT_sb = inp.tile([128, NKD, Q], bf16, tag="qTin")
                    refT = inp.tile([128, NKD, R], bf16, tag="refT")
                    for kd in range(NKD):
                        nc.sync.dma_start(
                            out=posT[:, kd, :],
                            in_=posTin[kd * 128 : (kd + 1) * 128, :],
                        )
                    for kd in range(NKD):
                        nc.sync.dma_start(
                            out=qT_sb[:, kd, :],
                            in_=qTin[kd * 128 : (kd + 1) * 128, :],
                        )
                    for kd in range(NKD):
                        nc.sync.dma_start(
                            out=refT[:, kd, :],
                            in_=refTin[kd * 128 : (kd + 1) * 128, :],
                        )

                    def load_w(pool, w_dram, tag):
                        tiles = []
                        for kd in range(NKD):
                            wt = pool.tile([128, D], bf16, tag=tag, name=tag)
                            nc.gpsimd.dma_start(
                                out=wt, in_=w_dram[kd * 128 : (kd + 1) * 128, :]
                            )
                            tiles.append(wt)
                        return tiles

                    with tc.tile_pool(name="wstA", bufs=8) as wstA:
                        # --- rT projection (weight tile reused over nb) ---
                        wr_t = load_w(wstA, Wr_d, "wr")
                        for i in range(NI):
                            ps = [
                                pj.tile([128, 512], f32, tag="pj", name="pj")
                                for _ in range(2)
                            ]
                            for kd in range(NKD):
                                for nb in range(2):
                                    nc.tensor.matmul(
                                        ps[nb],
                                        lhsT=wr_t[kd][
                                            :, i * 128 : (i + 1) * 128
                                        ],
                                        rhs=posT[:, kd, ds(nb * 512, 512)],
                                        start=(kd == 0),
                                        stop=(kd == NKD - 1),
                                    )
                            for nb in range(2):
                                eng = nc.vector if (i + nb) % 2 else nc.scalar
                                ecopy(eng, rT[:, i, ds(nb * 512, 512)], ps[nb])

                        # --- q projection -> qcb/qpb ---
                        wq_t = load_w(wstA, Wq_d, "wq")
                        for i in range(NI):
                            ps = pj.tile([128, 512], f32, tag="pj", name="pj")
                            for kd in range(NKD):
                                nc.tensor.matmul(
                                    ps,
                                    lhsT=wq_t[kd][:, i * 128 : (i + 1) * 128],
                                    rhs=qT_sb[:, kd, :],
                                    start=(kd == 0),
                                    stop=(kd == NKD - 1),
                                )
                            nc.vector.tensor_scalar_add(
                                qcb[:, i, :], ps, cb_sb[:, i : i + 1]
                            )
                            nc.vector.tensor_scalar_add(
                                qpb[:, i, :], ps, pb_sb[:, i : i + 1]
                            )

                    # --- prime pst pool pad tails (persist across reuse) ---
                    for _ in range(8):
                        t = pstp.tile([128, L], bf16, tag="pst", name="pst")
                        nc.vector.memset(t[:, R:L], PAD_VAL)

                    wstB = tc.alloc_tile_pool(name="wstB", bufs=8)
                    wk_t = load_w(wstB, Wk_d, "wk")
                    wv_t = load_w(wstB, Wv_d, "wv")

                    cp_cnt = [0]
                    cp_engs = [nc.vector, nc.scalar]

                    def emit_C(j):
                        """position logits P for heads 2j,2j+1 -> Pdram."""
                        for qt in range(NQT):
                            c0 = C0_QT[qt]
                            psts = [
                                pstp.tile([128, L], bf16, tag="pst", name="pst")
                                for _ in range(2)
                            ]
                            for rb in range(2):
                                pps = []
                                for hh in range(2):
                                    off = hh * 64
                                    pp = ppsum.tile([128, 512], f32,
                                                    tag="pp", name="pp")
                                    nc.tensor.matmul(
                                        pp,
                                        lhsT=qpb[off : off + 64, j,
                                                 ds(qt * 128, 128)],
                                        rhs=rT[off : off + 64, j,
                                               ds(rb * 512, 512)],
                                        start=True,
                                        stop=True,
                                        tile_position=(off, 0),
                                    )
                                    pps.append(pp)
                                for hh in range(2):
                                    lo = c0 if rb == 0 else 512
                                    eng = cp_engs[cp_cnt[0] % 2]
                                    cp_cnt[0] += 1
                                    ecopy(
                                        eng,
                                        psts[hh][:, lo : (rb + 1) * 512],
                                        pps[hh][:, lo - rb * 512 : 512],
                                    )
                            for hh in range(2):
                                h = 2 * j + hh
                                wr_ap = bass_mod.AP(
                                    tensor=Pdram[h].tensor,
                                    offset=Pdram[h].offset + qt * 128 * L + c0,
                                    ap=[[L, 128], [1, L - c0]],
                                )
                                nc.sync.dma_start(
                                    out=wr_ap, in_=psts[hh][:, c0:L]
                                )

                    # --- interleave C with kT / v projections ---
                    try:
                        for j in range(NP):
                            emit_C(j)
                            # kT block i=j
                            i = j
                            ps = [
                                pj.tile([128, 512], f32, tag="pj", name="pj")
                                for _ in range(2)
                            ]
                            for kd in range(NKD):
                                for nb in range(2):
                                    nc.tensor.matmul(
                                        ps[nb],
                                        lhsT=wk_t[kd][
                                            :, i * 128 : (i + 1) * 128
                                        ],
                                        rhs=refT[:, kd, ds(nb * 512, 512)],
                                        start=(kd == 0),
                                        stop=(kd == NKD - 1),
                                    )
                            for nb in range(2):
                                eng = nc.vector if nb else nc.scalar
                                ecopy(eng, kT[:, i, ds(nb * 512, 512)], ps[nb])
                            # v block rt=j
                            rt = j
                            ps = [
                                pj.tile([128, 512], f32, tag="pj", name="pj")
                                for _ in range(2)
                            ]
                            for kd in range(NKD):
                                for nb in range(2):
                                    nc.tensor.matmul(
                                        ps[nb],
                                        lhsT=refT[:, kd, ds(rt * 128, 128)],
                                        rhs=wv_t[kd][:, ds(nb * 512, 512)],
                                        start=(kd == 0),
                                        stop=(kd == NKD - 1),
                                    )
                            for nb in range(2):
                                eng = nc.scalar if nb else nc.vector
                                ecopy(
                                    eng, v_sb[:, rt, ds(nb * 512, 512)], ps[nb]
                                )
                    finally:
                        wstB.release()

            # ====== phases D/PV: software-pipelined over head pairs ======
            with (
                tc.tile_pool(name="psh", bufs=20) as pshp,
                tc.tile_pool(name="attn", bufs=4) as attnp,
                tc.tile_pool(name="attnT", bufs=4) as attnTp,
                tc.tile_pool(name="den", bufs=8) as denp,
                tc.tile_pool(name="wo", bufs=1) as wop,
                tc.tile_pool(name="scp", bufs=3, space="PSUM") as scp,
                tc.tile_pool(name="pvp", bufs=2, space="PSUM") as pvp,
            ):
                Wo_sb = wop.tile([128, NI, D], bf16, tag="Wo")
                nc.sync.dma_start(
                    out=Wo_sb, in_=Wo_d.rearrange("(i p) d -> p i d", p=128)
                )

                def emit_PSH(j):
                    """prefetch skewed (rel-shifted) P reads for pair j."""
                    pshs = {}
                    for qt in range(NQT):
                        w = W_QT[qt]
                        for hh in range(2):
                            h = 2 * j + hh
                            psh = pshp.tile([128, R], bf16, tag="psh",
                                            name="psh")
                            rd_ap = bass_mod.AP(
                                tensor=Pdram[h].tensor,
                                offset=Pdram[h].offset
                                + qt * 128 * (L - 1) + 511,
                                ap=[[L - 1, 128], [1, w]],
                            )
                            nc.gpsimd.dma_start(out=psh[:, 0:w], in_=rd_ap)
                            pshs[(hh, qt)] = psh
                    return pshs

                def emit_D(j, pshs):
                    """scores+exp+normalize+transpose for pair j."""
                    apair = {}
                    a32 = {}
                    for hh in range(2):
                        apair[hh] = attnp.tile([128, NQT, R], bf16,
                                               tag="attn", name="attn")
                        a32[hh] = attnTp.tile([128, NQT * NRT, 128], bf16,
                                              tag="attnT", name="attnT")
                        # zero fully-masked transposed blocks (rt >= qt+5)
                        for qt in range(NQT):
                            for rt in range(qt + 5, NRT):
                                nc.vector.memset(
                                    a32[hh][:, qt * NRT + rt, :], 0.0
                                )
                    for qt in range(NQT):
                        w = W_QT[qt]
                        scs = []
                        for hh in range(2):
                            sc = scp.tile([128, 1024], f32, tag="sc",
                                          name="sc")
                            scs.append(sc)
                        for rb in range(2):
                            hi = 512 if rb == 0 else w
                            for hh in range(2):
                                off = hh * 64
                                nc.tensor.matmul(
                                    scs[hh][:, rb * 512 : hi],
                                    lhsT=qcb[off : off + 64, j,
                                             ds(qt * 128, 128)],
                                    rhs=kT[off : off + 64, j, rb * 512 : hi],
                                    start=True,
                                    stop=False,
                                    tile_position=(off, 0),
                                )
                        for hh in range(2):
                            sc = scs[hh]
                            psh = pshs[(hh, qt)]
                            for rb in range(2):
                                hi = 512 if rb == 0 else w
                                nc.tensor.matmul(
                                    sc[:, rb * 512 : hi],
                                    lhsT=ident,
                                    rhs=psh[:, rb * 512 : hi],
                                    start=False,
                                    stop=True,
                                    skip_group_check=True,
                                )
                        for hh in range(2):
                            den = denp.tile([128, 4], f32, tag="den",
                                            name="den")
                            nc.scalar.activation(
                                apair[hh][:, qt, 0:w],
                                scs[hh][:, 0:w],
                                AF.Exp,
                                scale=0.125,
                                accum_out=den[:, 0:1],
                            )
                            nc.vector.reciprocal(den[:, 1:2], den[:, 0:1])
                            nc.vector.tensor_scalar_mul(
                                apair[hh][:, qt, 0:w],
                                apair[hh][:, qt, 0:w],
                                den[:, 1:2],
                            )
                            nc.sync.dma_start_transpose(
                                a32[hh][:, qt * NRT : qt * NRT + w // 128, :],
                                apair[hh][:, qt, 0:w],
                            )
                    aT = {}
                    for hh in range(2):
                        aT[hh] = a32[hh].rearrange(
                            "p (qt rt) q -> p qt rt q", rt=NRT
                        )
                    return aT

                def emit_PV(j, aT):
                    pv = pvp.tile([128, 512], f32, tag="pv", name="pv")
                    for rt in range(NRT):
                        for hh in range(2):
                            h = 2 * j + hh
                            off = hh * 64
                            nc.tensor.matmul(
                                pv[off : off + 64, :],
                                lhsT=v_sb[:, rt, ds(h * 64, 64)],
                                rhs=aT[hh][:, :, rt, :],
                                start=(rt == 0),
                                stop=(rt == NRT - 1),
                                tile_position=(0, off),
                            )
                    nc.vector.tensor_copy(outT[:, j, :], pv)

                pend = {}
                pshq = {0: emit_PSH(0), 1: emit_PSH(1)}
                for j in range(NP):
                    if j + 2 < NP:
                        pshq[j + 2] = emit_PSH(j + 2)
                    pend[j] = emit_D(j, pshq.pop(j))
                    if j - 1 in pend:
                        emit_PV(j - 1, pend.pop(j - 1))
                emit_PV(NP - 1, pend.pop(NP - 1))

            # ============ output projection ============
            with (
                tc.tile_pool(name="ost", bufs=4) as ostp,
                tc.tile_pool(name="opj", bufs=4, space="PSUM") as opj,
            ):
                for qt in range(NQT):
                    ops = [
                        opj.tile([128, 512], f32, tag="op", name="op")
                        for _ in range(2)
                    ]
                    for i in range(NI):
                        for db in range(2):
                            nc.tensor.matmul(
                                ops[db],
                                lhsT=outT[:, i, ds(qt * 128, 128)],
                                rhs=Wo_sb[:, i, ds(db * 512, 512)],
                                start=(i == 0),
                                stop=(i == NI - 1),
                            )
                    for db in range(2):
                        ot = ostp.tile([128, 512], f32, tag="ot", name="ot")
                        nc.vector.tensor_copy(ot, ops[db])
                        nc.scalar.dma_start(
                            out=out_d[
                                qt * 128 : (qt + 1) * 128,
                                db * 512 : (db + 1) * 512,
                            ],
                            in_=ot,
                        )

    return nc


def _get_nc():
    if "nc" not in _CACHE:
        nc = _build_nc()
        if not nc.is_finalized():
            nc.finalize()
        _CACHE["nc"] = nc
    return _CACHE["nc"]


def _prep_in_maps(inputs):
    import ml_dtypes

    bf = ml_dtypes.bfloat16
    q = np.asarray(inputs["query_seqs"], dtype=np.float32)
    mem = np.asarray(inputs["memory_seqs"], dtype=np.float32)
    pos = np.asarray(inputs["positional_encoding"], dtype=np.float32)
    Wq = np.asarray(inputs["Wq"], dtype=np.float32).reshape(D, D).astype(bf)
    Wk = np.asarray(inputs["Wk"], dtype=np.float32).reshape(D, D).astype(bf)
    Wv = np.asarray(inputs["Wv"], dtype=np.float32).reshape(D, D).astype(bf)
    Wr = np.asarray(inputs["Wr"], dtype=np.float32).reshape(D, D).astype(bf)
    Wo = np.asarray(inputs["Wo"], dtype=np.float32).reshape(D, D).astype(bf)
    cb = np.ascontiguousarray(
        np.asarray(inputs["content_bias"], dtype=np.float32)
        .reshape(D).reshape(NI, 128).T
    )
    pb = np.ascontiguousarray(
        np.asarray(inputs["position_bias"], dtype=np.float32)
        .reshape(D).reshape(NI, 128).T
    )
    posT = np.ascontiguousarray(pos.T).astype(bf)

    in_maps = []
    for b in range(B):
        refT = np.ascontiguousarray(
            np.concatenate([mem[b], q[b]], axis=0).T
        ).astype(bf)
        qT = np.ascontiguousarray(q[b].T).astype(bf)
        in_maps.append(
            dict(
                qT=qT, refT=refT, posT=posT,
                Wq=Wq, Wk=Wk, Wv=Wv, Wr=Wr, Wo=Wo, cb=cb, pb=pb,
            )
        )
    return in_maps


def run_spmd(inputs, **kwargs):
    """Run on 8 cores; returns (output [B,Q,D], BassKernelResults)."""
    from concourse.bass_utils import run_bass_kernel_spmd

    nc = _get_nc()
    in_maps = _prep_in_maps(inputs)
    res = run_bass_kernel_spmd(nc, in_maps, core_ids=list(range(B)), **kwargs)
    out = np.stack([r["out"] for r in res.results], axis=0).astype(np.float32)
    return out, res


def kernel(**inputs) -> np.ndarray:
    out, _ = run_spmd(inputs)
    return out


# revision 10
# speedup vs baseline: 1.4991x; 1.3188x over previous
"""TransformerXL relative attention on 8 TRN2 NeuronCores (batch-parallel).

v3: dense-pipeline rewrite of v2.
  - position-logit phase (C) interleaved with kT/v projections so TensorE
    stays warm (no HAM re-throttle) and P-scratch DMA overlaps matmuls
  - P scratch pitch L=1152: 128-col pad tail lives inside each pst tile
    (memset once per pool buffer) and is written by the same DMA as the
    data -> no separate pad-fill DMAs
  - masked-width trimming: per q-tile qt only cols [0, 640+128*qt) of the
    shifted scores are real; P writes [c0,1152), skewed reads [0,W),
    exp/normalize/transpose trimmed to W; fully-masked attnT blocks are
    memset to zero instead of transposed
  - engine split: Scalar=exp(+accum), Vector=normalize/memsets/copies,
    GpSimd=psh reads+copies, Sync=P writes+attn transposes, weights on
    GpSimd; deep psh prefetch (2 head-pairs ahead)
"""

import sys

if "/opt/trn_rl_repo" not in sys.path:
    sys.path.insert(0, "/opt/trn_rl_repo")

import numpy as np

B, Q, MEM, D, H, S = 8, 512, 512, 1024, 16, 64
R = Q + MEM  # 1024
L = R + 512  # 1536: padded row pitch of P scratch (1024 data + 512 pad)
PAD_VAL = -30000.0
NKD = D // 128  # 8 contraction tiles
NI = D // 128  # 8 hs-tiles
NQT = Q // 128  # 4 q-tiles
NRT = R // 128  # 8 r-tiles
NP = H // 2  # 8 head pairs
# per q-tile qt: shifted cols [0, W) are live; P data cols [c0, 1024) used
W_QT = [640, 768, 896, 1024]
C0_QT = [384, 256, 128, 0]

_CACHE = {}


def _build_nc():
    import concourse.bass as bass_mod
    import concourse.mybir as mybir
    import concourse.tile as tile
    from concourse import bacc
    from concourse.bass import ds
    from concourse.masks import make_identity

    f32 = mybir.dt.float32
    bf16 = mybir.dt.bfloat16
    AF = mybir.ActivationFunctionType

    nc = bacc.Bacc("TRN2", target_bir_lowering=False)

    qTin = nc.dram_tensor("qT", [D, Q], bf16, kind="ExternalInput")
    refTin = nc.dram_tensor("refT", [D, R], bf16, kind="ExternalInput")
    posTin = nc.dram_tensor("posT", [D, R], bf16, kind="ExternalInput")
    Wq_d = nc.dram_tensor("Wq", [D, D], bf16, kind="ExternalInput")
    Wk_d = nc.dram_tensor("Wk", [D, D], bf16, kind="ExternalInput")
    Wv_d = nc.dram_tensor("Wv", [D, D], bf16, kind="ExternalInput")
    Wr_d = nc.dram_tensor("Wr", [D, D], bf16, kind="ExternalInput")
    Wo_d = nc.dram_tensor("Wo", [D, D], bf16, kind="ExternalInput")
    cb_d = nc.dram_tensor("cb", [128, NI], f32, kind="ExternalInput")
    pb_d = nc.dram_tensor("pb", [128, NI], f32, kind="ExternalInput")
    out_d = nc.dram_tensor("out", [Q, D], f32, kind="ExternalOutput")


    def ecopy(eng, out, in_):
        if eng is nc.scalar:
            eng.copy(out, in_)
        else:
            eng.tensor_copy(out, in_)

    with tile.TileContext(nc) as tc:
        with (
            tc.tile_pool(name="persist", bufs=1) as persist,
            tc.tile_pool(name="dram", bufs=1, space="DRAM") as dram,
        ):
            ident = persist.tile([128, 128], bf16, tag="ident")
            make_identity(nc, ident)
            cb_sb = persist.tile([128, NI], f32, tag="cb")
            pb_sb = persist.tile([128, NI], f32, tag="pb")
            nc.sync.dma_start(out=cb_sb, in_=cb_d[:, :])
            nc.sync.dma_start(out=pb_sb, in_=pb_d[:, :])

            kT = persist.tile([128, NI, R], bf16, tag="kT")
            v_sb = persist.tile([128, NRT, D], bf16, tag="v")
            qcb = persist.tile([128, NI, Q], bf16, tag="qcb")
            qpb = persist.tile([128, NI, Q], bf16, tag="qpb")
            outT = persist.tile([128, NI, Q], bf16, tag="outT")

            Pdram = [
                dram.tile([Q * L], bf16, tag=f"pbuf{h}", name=f"pbuf{h}")
                for h in range(H)
            ]

            with tc.tile_pool(name="rtp", bufs=1) as rtp:
                rT = rtp.tile([128, NI, R], bf16, tag="rT")

                # ============ inputs + pre-phase: rT, qcb/qpb ============
                with (
                    tc.tile_pool(name="inp", bufs=1) as inp,
                    tc.tile_pool(name="pj", bufs=4, space="PSUM") as pj,
                    tc.tile_pool(name="pstp", bufs=8) as pstp,
                    tc.tile_pool(name="ppsum", bufs=4, space="PSUM") as ppsum,
                ):
                    posT = inp.tile([128, NKD, R], bf16, tag="posT")
                    qT_sb = inp.tile([128, NKD, Q], bf16, tag="qTin")
                    refT = inp.tile([128, NKD, R], bf16, tag="refT")
                    for kd in range(NKD):
                        nc.sync.dma_start(
                            out=posT[:, kd, :],
                            in_=posTin[kd * 128 : (kd + 1) * 128, :],
                        )
                    for kd in range(NKD):
                        nc.sync.dma_start(
                            out=qT_sb[:, kd, :],
                            in_=qTin[kd * 128 : (kd + 1) * 128, :],
                        )
                    for kd in range(NKD):
                        nc.sync.dma_start(
                            out=refT[:, kd, :],
                            in_=refTin[kd * 128 : (kd + 1) * 128, :],
                        )

                    def load_w(pool, w_dram, tag):
                        tiles = []
                        for kd in range(NKD):
                            wt = pool.tile([128, D], bf16, tag=tag, name=tag)
                            nc.gpsimd.dma_start(
                                out=wt, in_=w_dram[kd * 128 : (kd + 1) * 128, :]
                            )
                            tiles.append(wt)
                        return tiles

                    with tc.tile_pool(name="wstA", bufs=8) as wstA:
                        # --- rT projection (weight tile reused over nb) ---
                        wr_t = load_w(wstA, Wr_d, "wr")
                        for i in range(NI):
                            ps = [
                                pj.tile([128, 512], f32, tag="pj", name="pj")
                                for _ in range(2)
                            ]
                            for kd in range(NKD):
                                for nb in range(2):
                                    nc.tensor.matmul(
                                        ps[nb],
                                        lhsT=wr_t[kd][
                                            :, i * 128 : (i + 1) * 128
                                        ],
                                        rhs=posT[:, kd, ds(nb * 512, 512)],
                                        start=(kd == 0),
                                        stop=(kd == NKD - 1),
                                    )
                            for nb in range(2):
                                eng = nc.vector if (i + nb) % 2 else nc.scalar
                                ecopy(eng, rT[:, i, ds(nb * 512, 512)], ps[nb])

                        # --- q projection -> qcb/qpb ---
                        wq_t = load_w(wstA, Wq_d, "wq")
                        for i in range(NI):
                            ps = pj.tile([128, 512], f32, tag="pj", name="pj")
                            for kd in range(NKD):
                                nc.tensor.matmul(
                                    ps,
                                    lhsT=wq_t[kd][:, i * 128 : (i + 1) * 128],
                                    rhs=qT_sb[:, kd, :],
                                    start=(kd == 0),
                                    stop=(kd == NKD - 1),
                                )
                            nc.vector.tensor_scalar_add(
                                qcb[:, i, :], ps, cb_sb[:, i : i + 1]
                            )
                            nc.vector.tensor_scalar_add(
                                qpb[:, i, :], ps, pb_sb[:, i : i + 1]
                            )

                    # --- prime pst pool pad tails (persist across reuse) ---
                    for _ in range(8):
                        t = pstp.tile([128, L], bf16, tag="pst", name="pst")
                        nc.vector.memset(t[:, R:L], PAD_VAL)

                    wstB = tc.alloc_tile_pool(name="wstB", bufs=8)
                    wk_t = load_w(wstB, Wk_d, "wk")
                    wv_t = load_w(wstB, Wv_d, "wv")

                    cp_cnt = [0]
                    cp_engs = [nc.vector, nc.scalar]

                    def emit_C(j):
                        """position logits P for heads 2j,2j+1 -> Pdram."""
                        for qt in range(NQT):
                            c0 = C0_QT[qt]
                            psts = [
                                pstp.tile([128, L], bf16, tag="pst", name="pst")
                                for _ in range(2)
                            ]
                            for rb in range(2):
                                pps = []
                                for hh in range(2):
                                    off = hh * 64
                                    pp = ppsum.tile([128, 512], f32,
                                                    tag="pp", name="pp")
                                    nc.tensor.matmul(
                                        pp,
                                        lhsT=qpb[off : off + 64, j,
                                                 ds(qt * 128, 128)],
                                        rhs=rT[off : off + 64, j,
                                               ds(rb * 512, 512)],
                                        start=True,
                                        stop=True,
                                        tile_position=(off, 0),
                                    )
                                    pps.append(pp)
                                for hh in range(2):
                                    lo = c0 if rb == 0 else 512
                                    eng = cp_engs[cp_cnt[0] % 2]
                                    cp_cnt[0] += 1
                                    ecopy(
                                        eng,
                                        psts[hh][:, lo : (rb + 1) * 512],
                                        pps[hh][:, lo - rb * 512 : 512],
                                    )
                            for hh in range(2):
                                h = 2 * j + hh
                                wr_ap = bass_mod.AP(
                                    tensor=Pdram[h].tensor,
                                    offset=Pdram[h].offset + qt * 128 * L + c0,
                                    ap=[[L, 128], [1, L - c0]],
                                )
                                nc.sync.dma_start(
                                    out=wr_ap, in_=psts[hh][:, c0:L]
                                )

                    # --- interleave C with kT / v projections ---
                    try:
                        for j in range(NP):
                            emit_C(j)
                            # kT block i=j
                            i = j
                            ps = [
                                pj.tile([128, 512], f32, tag="pj", name="pj")
                                for _ in range(2)
                            ]
                            for kd in range(NKD):
                                for nb in range(2):
                                    nc.tensor.matmul(
                                        ps[nb],
                                        lhsT=wk_t[kd][
                                            :, i * 128 : (i + 1) * 128
                                        ],
                                        rhs=refT[:, kd, ds(nb * 512, 512)],
                                        start=(kd == 0),
                                        stop=(kd == NKD - 1),
                                    )
                            for nb in range(2):
                                eng = nc.vector if nb else nc.scalar
                                ecopy(eng, kT[:, i, ds(nb * 512, 512)], ps[nb])
                            # v block rt=j
                            rt = j
                            ps = [
                                pj.tile([128, 512], f32, tag="pj", name="pj")
                                for _ in range(2)
                            ]
                            for kd in range(NKD):
                                for nb in range(2):
                                    nc.tensor.matmul(
                                        ps[nb],
                                        lhsT=refT[:, kd, ds(rt * 128, 128)],
                                        rhs=wv_t[kd][:, ds(nb * 512, 512)],
                                        start=(kd == 0),
                                        stop=(kd == NKD - 1),
                                    )
                            for nb in range(2):
                                eng = nc.scalar if nb else nc.vector
                                ecopy(
                                    eng, v_sb[:, rt, ds(nb * 512, 512)], ps[nb]
                                )
                    finally:
                        wstB.release()

            # ====== phases D/PV: software-pipelined over head pairs ======
            with (
                tc.tile_pool(name="psh", bufs=6) as pshp,
                tc.tile_pool(name="attn", bufs=4) as attnp,
                tc.tile_pool(name="attnT", bufs=4) as attnTp,
                tc.tile_pool(name="den", bufs=8) as denp,
                tc.tile_pool(name="wo", bufs=1) as wop,
                tc.tile_pool(name="scp", bufs=3, space="PSUM") as scp,
                tc.tile_pool(name="pvp", bufs=2, space="PSUM") as pvp,
            ):
                Wo_sb = wop.tile([128, NI, D], bf16, tag="Wo")
                nc.sync.dma_start(
                    out=Wo_sb, in_=Wo_d.rearrange("(i p) d -> p i d", p=128)
                )

                def emit_PSH(j):
                    """prefetch skewed (rel-shifted) P reads for pair j."""
                    pshs = {}
                    for hh in range(2):
                        h = 2 * j + hh
                        psh = pshp.tile([128, NQT, R], bf16, tag="psh",
                                        name="psh")
                        rd_ap = bass_mod.AP(
                            tensor=Pdram[h].tensor,
                            offset=Pdram[h].offset + 511,
                            ap=[[L - 1, 128], [128 * (L - 1), NQT], [1, R]],
                        )
                        nc.gpsimd.dma_start(out=psh, in_=rd_ap)
                        pshs[hh] = psh
                    return pshs

                def emit_D(j, pshs):
                    """scores+exp+normalize+transpose for pair j."""
                    apair = {}
                    a32 = {}
                    for hh in range(2):
                        apair[hh] = attnp.tile([128, NQT, R], bf16,
                                               tag="attn", name="attn")
                        a32[hh] = attnTp.tile([128, NQT * NRT, 128], bf16,
                                              tag="attnT", name="attnT")
                        # zero masked attn tails so the full-width transpose
                        # lands zeros in masked attnT blocks
                        for qt in range(NQT - 1):
                            nc.vector.memset(
                                apair[hh][:, qt, W_QT[qt] : R], 0.0
                            )
                    for qt in range(NQT):
                        w = W_QT[qt]
                        scs = []
                        for hh in range(2):
                            sc = scp.tile([128, 1024], f32, tag="sc",
                                          name="sc")
                            scs.append(sc)
                        for rb in range(2):
                            hi = 512 if rb == 0 else w
                            for hh in range(2):
                                off = hh * 64
                                nc.tensor.matmul(
                                    scs[hh][:, rb * 512 : hi],
                                    lhsT=qcb[off : off + 64, j,
                                             ds(qt * 128, 128)],
                                    rhs=kT[off : off + 64, j, rb * 512 : hi],
                                    start=True,
                                    stop=False,
                                    tile_position=(off, 0),
                                )
                        for hh in range(2):
                            sc = scs[hh]
                            for rb in range(2):
                                hi = 512 if rb == 0 else w
                                nc.tensor.matmul(
                                    sc[:, rb * 512 : hi],
                                    lhsT=ident,
                                    rhs=pshs[hh][:, qt, rb * 512 : hi],
                                    start=False,
                                    stop=True,
                                    skip_group_check=True,
                                )
                        for hh in range(2):
                            den = denp.tile([128, 4], f32, tag="den",
                                            name="den")
                            nc.scalar.activation(
                                apair[hh][:, qt, 0:w],
                                scs[hh][:, 0:w],
                                AF.Exp,
                                scale=0.125,
                                accum_out=den[:, 0:1],
                            )
                            nc.vector.reciprocal(den[:, 1:2], den[:, 0:1])
                            nc.vector.tensor_scalar_mul(
                                apair[hh][:, qt, 0:w],
                                apair[hh][:, qt, 0:w],
                                den[:, 1:2],
                            )
                    for hh in range(2):
                        nc.sync.dma_start_transpose(
                            a32[hh], apair[hh].rearrange("p a b -> p (a b)")
                        )
                    aT = {}
                    for hh in range(2):
                        aT[hh] = a32[hh].rearrange(
                            "p (qt rt) q -> p qt rt q", rt=NRT
                        )
                    return aT

                def emit_PV(j, aT):
                    pv = pvp.tile([128, 512], f32, tag="pv", name="pv")
                    for rt in range(NRT):
                        for hh in range(2):
                            h = 2 * j + hh
                            off = hh * 64
                            nc.tensor.matmul(
                                pv[off : off + 64, :],
                                lhsT=v_sb[:, rt, ds(h * 64, 64)],
                                rhs=aT[hh][:, :, rt, :],
                                start=(rt == 0),
                                stop=(rt == NRT - 1),
                                tile_position=(0, off),
                            )
                    nc.vector.tensor_copy(outT[:, j, :], pv)

                pend = {}
                pshq = {0: emit_PSH(0), 1: emit_PSH(1)}
                for j in range(NP):
                    if j + 2 < NP:
                        pshq[j + 2] = emit_PSH(j + 2)
                    pend[j] = emit_D(j, pshq.pop(j))
                    if j - 1 in pend:
                        emit_PV(j - 1, pend.pop(j - 1))
                emit_PV(NP - 1, pend.pop(NP - 1))

            # ============ output projection ============
            with (
                tc.tile_pool(name="ost", bufs=4) as ostp,
                tc.tile_pool(name="opj", bufs=4, space="PSUM") as opj,
            ):
                for qt in range(NQT):
                    ops = [
                        opj.tile([128, 512], f32, tag="op", name="op")
                        for _ in range(2)
                    ]
                    for i in range(NI):
                        for db in range(2):
                            nc.tensor.matmul(
                                ops[db],
                                lhsT=outT[:, i, ds(qt * 128, 128)],
                                rhs=Wo_sb[:, i, ds(db * 512, 512)],
                                start=(i == 0),
                                stop=(i == NI - 1),
                            )
                    for db in range(2):
                        ot = ostp.tile([128, 512], f32, tag="ot", name="ot")
                        nc.vector.tensor_copy(ot, ops[db])
                        nc.scalar.dma_start(
                            out=out_d[
                                qt * 128 : (qt + 1) * 128,
                                db * 512 : (db + 1) * 512,
                            ],
                            in_=ot,
                        )

    return nc


def _get_nc():
    if "nc" not in _CACHE:
        nc = _build_nc()
        if not nc.is_finalized():
            nc.finalize()
        _CACHE["nc"] = nc
    return _CACHE["nc"]


def _prep_in_maps(inputs):
    import ml_dtypes

    bf = ml_dtypes.bfloat16
    q = np.asarray(inputs["query_seqs"], dtype=np.float32)
    mem = np.asarray(inputs["memory_seqs"], dtype=np.float32)
    pos = np.asarray(inputs["positional_encoding"], dtype=np.float32)
    Wq = np.asarray(inputs["Wq"], dtype=np.float32).reshape(D, D).astype(bf)
    Wk = np.asarray(inputs["Wk"], dtype=np.float32).reshape(D, D).astype(bf)
    Wv = np.asarray(inputs["Wv"], dtype=np.float32).reshape(D, D).astype(bf)
    Wr = np.asarray(inputs["Wr"], dtype=np.float32).reshape(D, D).astype(bf)
    Wo = np.asarray(inputs["Wo"], dtype=np.float32).reshape(D, D).astype(bf)
    cb = np.ascontiguousarray(
        np.asarray(inputs["content_bias"], dtype=np.float32)
        .reshape(D).reshape(NI, 128).T
    )
    pb = np.ascontiguousarray(
        np.asarray(inputs["position_bias"], dtype=np.float32)
        .reshape(D).reshape(NI, 128).T
    )
    posT = np.ascontiguousarray(pos.T).astype(bf)

    in_maps = []
    for b in range(B):
        refT = np.ascontiguousarray(
            np.concatenate([mem[b], q[b]], axis=0).T
        ).astype(bf)
        qT = np.ascontiguousarray(q[b].T).astype(bf)
        in_maps.append(
            dict(
                qT=qT, refT=refT, posT=posT,
                Wq=Wq, Wk=Wk, Wv=Wv, Wr=Wr, Wo=Wo, cb=cb, pb=pb,
            )
        )
    return in_maps


def run_spmd(inputs, **kwargs):
    """Run on 8 cores; returns (output [B,Q,D], BassKernelResults)."""
    from concourse.bass_utils import run_bass_kernel_spmd

    nc = _get_nc()
    in_maps = _prep_in_maps(inputs)
    res = run_bass_kernel_spmd(nc, in_maps, core_ids=list(range(B)), **kwargs)
    out = np.stack([r["out"] for r in res.results], axis=0).astype(np.float32)
    return out, res


def kernel(**inputs) -> np.ndarray:
    out, _ = run_spmd(inputs)
    return out
